# revision 7
# baseline (speedup 1.0000x reference)
"""AttentionFusion kernel for 8 TRN2 NeuronCores.

Reference computation:
    expanded_video = repeat_interleave(video, 20, dim=1)        # [B, 1280, D]
    scores = expanded_video @ text.T * D**-0.5                  # [B, 1280, 256]
    attn_out = softmax(scores) @ text                           # [B, 1280, D]
    out = concat([text, video, expanded_video + attn_out], 1)   # [B, 1600, D]

Key algebraic fact: repeated (identical) query rows produce identical
attention outputs, so only the 64 unique video rows per batch need
attention; the 20x replication happens on the host during unsharding.

Sharding (v6): one core PAIR per batch, split along the SOFTMAX K
dimension (256 text rows -> 128 per core), zero cross-core traffic.
Each core computes UNNORMALIZED attention over its own k-half:
    E_h = exp(scores_h * scale)      (no max subtraction; |s| < ~5)
    O_h = E_h @ T_h                  (unnormalized partial, full D)
    s_h = rowsum(E_h)
and the host combines:  attn = (O_0 + O_1) / (s_0 + s_1).
This removes the v5 redundant full-K stage-1 (the dominant PE cost:
fp8 matmul is power-throttled to ~half rate) and halves stage-1 work.

Stage 1 runs in fp8e4 (e4m3) with DoubleRow perf mode: 2 contraction
rows per partition + 2 rhs cols/cycle -> stage-1 is ~8x cheaper than
v5's throttled fp8e3, leaving the kernel input-DMA-bound. Stage 2
(values) runs in fp16 (same cost as bf16, 3 more mantissa bits).

Host pre-transposes inputs into the layouts the TensorEngine needs
(contraction dim on partitions), so every DMA is contiguous.
"""

import sys

import numpy as np

if "/opt/trn_rl_repo" not in sys.path:
    sys.path.insert(0, "/opt/trn_rl_repo")

import ml_dtypes

REPEAT = 20
D = 10240
SCALE = D ** (-0.5)
B, TT, TV = 4, 256, 64
KH = 128          # k-half: text rows per core
DJ = 40           # stage-1 contraction chunks (256 d each, DoubleRow)
QCH = 4           # qtt DMA chunks
NR = 10           # stage-2 rounds; each = 2 col groups x 512 cols
NCORES = 8

_compiled = None


def _build():
    import concourse.mybir as mybir
    import concourse.tile as tile
    from concourse import bacc
    from concourse.masks import make_identity

    f32 = mybir.dt.float32
    f16 = mybir.dt.float16
    fp8 = mybir.dt.float8e4

    nc = bacc.Bacc(
        "TRN2", target_bir_lowering=False, debug=False, num_devices=NCORES
    )
    qtt_h = nc.dram_tensor("qtt", [128, DJ, 2, TV + KH], fp8, kind="ExternalInput")
    tn_h = nc.dram_tensor("tn", [128, NR * 2, 512], f16, kind="ExternalInput")
    out_h = nc.dram_tensor("out", [128, NR, 512], f16, kind="ExternalOutput")
    ls_h = nc.dram_tensor("lsum", [TV, 1], f32, kind="ExternalOutput")

    JC = DJ // QCH    # stage-1 j's per qtt DMA chunk

    with tile.TileContext(nc) as tc:
        with (
            tc.tile_pool(name="qtp", bufs=QCH) as qt_pool,
            tc.tile_pool(name="tnp", bufs=2) as tn_pool,
            tc.tile_pool(name="smp", bufs=1) as sm_pool,
            tc.tile_pool(name="osp", bufs=NR // 2) as os_pool,
            tc.tile_pool(name="ps_1", bufs=1, space="PSUM") as ps_1_pool,
            tc.tile_pool(name="ps_w", bufs=1, space="PSUM") as ps_w_pool,
            tc.tile_pool(name="ps_o", bufs=4, space="PSUM") as ps_o_pool,
        ):
            ident = sm_pool.tile([TV, TV], f16, tag="ident")
            make_identity(nc, ident[:])

            # stage 1: S_h = Q @ T_h.T in fp8e4 DoubleRow (2 contraction
            # rows/partition, 2 cols/cycle; tile_position packing is not
            # legal ISA with DoubleRow, so one accumulation group)
            ps1 = ps_1_pool.tile([TV, KH], f32)
            for c in range(QCH):
                qsb = qt_pool.tile([128, JC, 2, TV + KH], fp8)
                nc.sync.dma_start(qsb[:], qtt_h[:, c * JC : (c + 1) * JC])
                for j in range(JC):
                    jj = c * JC + j
                    nc.tensor.matmul(
                        ps1[:],
                        lhsT=qsb[:, j, :, 0:TV],
                        rhs=qsb[:, j, :, TV : TV + KH],
                        start=(jj == 0),
                        stop=(jj == DJ - 1),
                        perf_mode=mybir.MatmulPerfMode.DoubleRow,
                    )

            # stage-2 operand streams in while stage 1 runs
            tn_sb = []
            for r in range(2):
                t = tn_pool.tile([128, NR, 512], f16)
                nc.sync.dma_start(t[:], tn_h[:, r * NR : (r + 1) * NR, :])
                tn_sb.append(t)

            # unnormalized exp: E = exp(S * scale), row sums via accumulator
            e_sb = sm_pool.tile([TV, KH], f16, tag="e")
            lsum = sm_pool.tile([TV, 1], f32, tag="lsum")
            nc.scalar.activation(
                e_sb[:],
                ps1[:],
                mybir.ActivationFunctionType.Exp,
                scale=SCALE,
                accum_out=lsum[:],
            )
            nc.scalar.dma_start(ls_h[:], lsum[:])

            # E[64, 128] -> ET[128, 64] (k on partitions) via PE transpose
            wt_ps = ps_w_pool.tile([KH, TV], f16)
            nc.tensor.transpose(wt_ps[:], e_sb[:], ident[:])
            wt_sb = sm_pool.tile([KH, TV], f16, tag="wt")
            nc.scalar.copy(wt_sb[:], wt_ps[:])

            # stage 2: O_h = E_h @ T_h, 2x column-tiled (same weights at
            # two tile positions, two rhs streams)
            osb = None
            for r in range(NR):
                ps_o = ps_o_pool.tile([128, 512], f32)
                for g in range(2):
                    n = 2 * r + g
                    nc.tensor.matmul(
                        ps_o[g * TV : (g + 1) * TV, :],
                        lhsT=wt_sb[:],
                        rhs=tn_sb[n // NR][:, n % NR, :],
                        start=True,
                        stop=True,
                        tile_position=(0, g * TV),
                        skip_group_check=True,
                    )
                if r % 2 == 0:
                    osb = os_pool.tile([128, 2, 512], f16)
                    nc.vector.tensor_copy(osb[:, 0, :], ps_o[:])
                else:
                    nc.scalar.copy(osb[:, 1, :], ps_o[:])
                    nc.gpsimd.dma_start(
                        out_h[:, r - 1 : r + 1, :], osb[:]
                    )

    nc.compile()
    return nc


def _prepare_in_maps(text, video):
    tf = np.asarray(text, dtype=np.float32)
    vf = np.asarray(video, dtype=np.float32)
    t8 = tf.astype(ml_dtypes.float8_e4m3)
    v8 = vf.astype(ml_dtypes.float8_e4m3)
    t16 = tf.astype(np.float16)
    in_maps = []
    for c in range(NCORES):
        b, h = divmod(c, 2)
        # qtt[p, j, i, q]      = video[b, q, j*256 + i*128 + p]
        # qtt[p, j, i, 64+kk]  = text[b, h*128 + kk, j*256 + i*128 + p]
        qtt = np.empty((128, DJ, 2, TV + KH), dtype=ml_dtypes.float8_e4m3)
        qtt[:, :, :, 0:TV] = (
            v8[b].reshape(TV, DJ, 2, 128).transpose(3, 1, 2, 0)
        )
        qtt[:, :, :, TV:] = (
            t8[b, h * KH : (h + 1) * KH]
            .reshape(KH, DJ, 2, 128)
            .transpose(3, 1, 2, 0)
        )
        # tn[p, n, c] = text[b, h*128 + p, n*512 + c]
        tn = np.ascontiguousarray(
            t16[b, h * KH : (h + 1) * KH].reshape(128, NR * 2, 512)
        )
        in_maps.append({"qtt": qtt, "tn": tn})
    return in_maps


def _assemble(results, text, video):
    tf = np.asarray(text, dtype=np.float32)
    vf = np.asarray(video, dtype=np.float32)
    onum = np.zeros((B, TV, D), np.float32)
    oden = np.zeros((B, TV, 1), np.float32)
    for c in range(NCORES):
        b, h = divmod(c, 2)
        o = np.asarray(results[c]["out"], dtype=np.float32)
        # out[64*g + q, r, x] = O_h[q, (2r+g)*512 + x]
        o = o.reshape(2, TV, NR, 512).transpose(1, 2, 0, 3).reshape(TV, D)
        onum[b] += o
        oden[b] += np.asarray(results[c]["lsum"], dtype=np.float32).reshape(
            TV, 1
        )
    fused = vf + onum / oden
    return np.concatenate([tf, vf, np.repeat(fused, REPEAT, axis=1)], axis=1)


def _ensure_ntff_hook():
    """Register the axon NTFF profiling hook if the image lacks
    antenv.axon_hooks (trace=True degrades to no-op otherwise)."""
    import types

    try:
        from antenv import axon_hooks  # noqa: F401

        return
    except ImportError:
        pass
    mod = types.ModuleType("antenv.axon_hooks")
    _hook = [None]
    mod.set_axon_ntff_profile_hook = lambda h: _hook.__setitem__(0, h)
    mod.get_axon_ntff_profile_hook = lambda: _hook[0]
    sys.modules["antenv.axon_hooks"] = mod
    import antenv

    antenv.axon_hooks = mod
    try:
        from trn_agent_boot.trn_boot import _ntff_profile_via_ctypes

        mod.set_axon_ntff_profile_hook(
            _ntff_profile_via_ctypes("/opt/axon/libaxon_pjrt.so")
        )
    except Exception:
        pass


def _run(text_features, video_features, trace=False, **spmd_kwargs):
    global _compiled
    if _compiled is None:
        _compiled = _build()
    if trace:
        _ensure_ntff_hook()
    from concourse.bass_utils import run_bass_kernel_spmd

    in_maps = _prepare_in_maps(text_features, video_features)
    res = run_bass_kernel_spmd(
        _compiled,
        in_maps,
        core_ids=list(range(NCORES)),
        trace=trace,
        **spmd_kwargs,
    )
    out = _assemble(res.results, text_features, video_features)
    return out, res


def kernel(text_features, video_features):
    out, _ = _run(text_features, video_features)
    return out
